# revision 16
# baseline (speedup 1.0000x reference)
"""Trainium2 Bass kernel for:
    out[b,c,h,w] = mean_w(x1[b,c,h,:]) * mean_h(avgpool2(x2)[b,c,:,w])

Math:
    rowsum1[b,c,h] = sum_w x1[b,c,h,w]                       (reduce over free axis, DVE)
    colsum2[b,c,w] = sum_h x2[b,c,h,w]                       (reduce over partitions, PE w/ ones)
    mean2p[b,c,w]  = (colsum2[b,c,2w] + colsum2[b,c,2w+1])   (pair-add, avgpool cols)
    out[b,c,h,w]   = rowsum1[h] * mean2p[w] / (256*1024)

Sharding: B (=16) split across 8 cores -> 2 B x 32 C = 64 (b,c) pairs per core.
All per-(b,c) work is independent; no collectives.
"""

import numpy as np
import concourse.bacc as bacc
import concourse.mybir as mybir
from concourse.tile import TileContext
from concourse.bass_utils import run_bass_kernel_spmd

N_CORES = 8
B, C, H, W = 16, 32, 256, 256
H2, W2 = 512, 512
B_LOC = B // N_CORES          # 2
BC = B_LOC * C                # 64 (b,c) pairs per core
X1_GRP = 8                    # (b,c) pairs per x1 load group
N_GRP = BC // X1_GRP
NJ = H // 128                 # 2 h-blocks per pair
NC2 = H2 // 128               # 4 h-blocks per x2 pair
SCALE = 1.0 / (256.0 * 1024.0)  # 2**-18: mean1 (/256) * mean2 (/4 pool * /256 rows)
F32 = mybir.dt.float32
F32R = mybir.dt.float32r      # fast fp32 matmul mode (1 cycle/row at N>=256)

# Use float32r for the x2 column-sum matmuls (4x PE speedup, slight rounding).
USE_F32R = True

_built = {}


def _build(reps=1):
    """Build the Bass program. reps>1 repeats the whole workload in-kernel
    (used only for benchmarking; results identical)."""
    if reps in _built:
        return _built[reps]

    nc = bacc.Bacc("TRN2", target_bir_lowering=False, debug=False,
                   num_devices=N_CORES)
    mm_dt = F32R if USE_F32R else F32
    x1 = nc.dram_tensor("x1", [BC * H, W], F32, kind="ExternalInput")
    x2 = nc.dram_tensor("x2", [BC * H2, W2], mm_dt, kind="ExternalInput")
    out = nc.dram_tensor("out", [BC * H, W], F32, kind="ExternalOutput")

    # Row-interleaved views: partition p <-> (row % 128) so per-partition
    # scalars line up with output row blocks.
    x1v = x1.ap().rearrange("(g j p) w -> g p j w", j=NJ * X1_GRP, p=128)
    x2v = x2.ap().rearrange("(n c p) w -> n p c w", c=NC2, p=128)
    outv = out.ap().rearrange("(n j p) w -> n p j w", j=NJ, p=128)

    with TileContext(nc) as tc:
        with (
            tc.tile_pool(name="const", bufs=1) as cpool,
            tc.tile_pool(name="x1p", bufs=2) as x1pool,
            tc.tile_pool(name="rsp", bufs=2) as rspool,
            tc.tile_pool(name="x2p", bufs=6) as x2pool,
            tc.tile_pool(name="csb", bufs=6) as csbpool,
            tc.tile_pool(name="m2p", bufs=6) as m2pool,
            tc.tile_pool(name="op", bufs=6) as opool,
            tc.tile_pool(name="csp", bufs=4, space="PSUM") as cspool,
            tc.tile_pool(name="pbp", bufs=4, space="PSUM") as pbpool,
        ):
            ones_col = cpool.tile([128, 1], mm_dt)
            if USE_F32R:
                ones_f32 = cpool.tile([128, 1], F32)
                nc.vector.memset(ones_f32[:], 1.0)
                nc.vector.tensor_copy(ones_col[:], ones_f32[:])
            else:
                nc.vector.memset(ones_col[:], 1.0)
            scale_row = cpool.tile([1, 128], F32)
            nc.vector.memset(scale_row[:], SCALE)

            for _rep in range(reps):
              for g in range(N_GRP):
                # x1 rowsums for X1_GRP pairs at once.
                x1t = x1pool.tile([128, NJ * X1_GRP, W], F32)
                nc.sync.dma_start(out=x1t[:], in_=x1v[g])
                rs = rspool.tile([128, NJ * X1_GRP], F32)
                nc.vector.reduce_sum(out=rs[:], in_=x1t[:],
                                     axis=mybir.AxisListType.X)

                for s in range(X1_GRP):
                    n = g * X1_GRP + s
                    x2t = x2pool.tile([128, NC2, W2], mm_dt)
                    nc.sync.dma_start(out=x2t[:], in_=x2v[n])

                    # colsum2 over all 512 rows -> PSUM (1, 512)
                    cs = cspool.tile([1, W2], F32)
                    for ci in range(NC2):
                        nc.tensor.matmul(
                            cs[:],
                            lhsT=ones_col[:],
                            rhs=x2t[:, ci, :],
                            start=(ci == 0),
                            stop=(ci == NC2 - 1),
                        )

                    # PSUM -> SBUF, then pair-add adjacent columns (avgpool).
                    csb = csbpool.tile([1, W2], F32)
                    nc.vector.tensor_copy(csb[:], cs[:])
                    m2 = m2pool.tile([1, W], F32)
                    csv = csb[:].rearrange("p (w t) -> p w t", t=2)
                    nc.vector.tensor_add(m2[:], csv[:, :, 0], csv[:, :, 1])

                    # Broadcast mean2 (scaled) to 128 partitions via K=1 matmul.
                    pb = pbpool.tile([128, W], F32)
                    nc.tensor.matmul(
                        pb[:],
                        lhsT=scale_row[:],
                        rhs=m2[:],
                        start=True,
                        stop=True,
                    )

                    # Outer product: scale each partition by rowsum1.
                    ot = opool.tile([128, NJ, W], F32)
                    for j in range(NJ):
                        col = NJ * s + j
                        nc.scalar.activation(
                            ot[:, j, :], pb[:],
                            mybir.ActivationFunctionType.Copy,
                            scale=rs[:, col:col + 1],
                        )
                    # Store via the scalar engine's DGE queue so stores don't
                    # head-of-line block the SP queue that issues loads.
                    nc.scalar.dma_start(out=outv[n], in_=ot[:])

    nc.compile()
    _built[reps] = nc
    return nc


def _in_maps(x1, x2):
    x1 = np.ascontiguousarray(np.asarray(x1), dtype=np.float32)
    x2 = np.ascontiguousarray(np.asarray(x2), dtype=np.float32)
    maps = []
    for i in range(N_CORES):
        maps.append({
            "x1": x1[i * B_LOC:(i + 1) * B_LOC].reshape(BC * H, W),
            "x2": x2[i * B_LOC:(i + 1) * B_LOC].reshape(BC * H2, W2),
        })
    return maps


def _run(x1, x2, **kw):
    nc = _build()
    return run_bass_kernel_spmd(nc, _in_maps(x1, x2), list(range(N_CORES)), **kw)


def kernel(x1, x2):
    res = _run(x1, x2)
    outs = [res.results[i]["out"].reshape(B_LOC, C, H, W)
            for i in range(N_CORES)]
    return np.concatenate(outs, axis=0)


# revision 19
# speedup vs baseline: 1.2279x; 1.2279x over previous
"""Trainium2 Bass kernel for:
    out[b,c,h,w] = mean_w(x1[b,c,h,:]) * mean_h(avgpool2(x2)[b,c,:,w])

Math:
    rowsum1[b,c,h] = sum_w x1[b,c,h,w]                       (reduce over free axis, DVE)
    colsum2[b,c,w] = sum_h x2[b,c,h,w]                       (reduce over partitions, PE w/ ones)
    mean2p[b,c,w]  = (colsum2[b,c,2w] + colsum2[b,c,2w+1])   (pair-add, avgpool cols)
    out[b,c,h,w]   = rowsum1[h] * mean2p[w] / (256*1024)

Sharding: B (=16) split across 8 cores -> 2 B x 32 C = 64 (b,c) pairs per core.
All per-(b,c) work is independent; no collectives.
"""

import numpy as np
import concourse.bacc as bacc
import concourse.mybir as mybir
from concourse.tile import TileContext
from concourse.bass_utils import run_bass_kernel_spmd

N_CORES = 8
B, C, H, W = 16, 32, 256, 256
H2, W2 = 512, 512
B_LOC = B // N_CORES          # 2
BC = B_LOC * C                # 64 (b,c) pairs per core
X1_GRP = 8                    # (b,c) pairs per x1 load group
N_GRP = BC // X1_GRP
NJ = H // 128                 # 2 h-blocks per pair
NC2 = H2 // 128               # 4 h-blocks per x2 pair
SCALE = 1.0 / (256.0 * 1024.0)  # 2**-18: mean1 (/256) * mean2 (/4 pool * /256 rows)
F32 = mybir.dt.float32
F32R = mybir.dt.float32r      # fast fp32 matmul mode (1 cycle/row at N>=256)

# Use float32r for the x2 column-sum matmuls (4x PE speedup, slight rounding).
USE_F32R = True

_built = {}


def _build(reps=1):
    """Build the Bass program. reps>1 repeats the whole workload in-kernel
    (used only for benchmarking; results identical)."""
    if reps in _built:
        return _built[reps]

    nc = bacc.Bacc("TRN2", target_bir_lowering=False, debug=False,
                   num_devices=N_CORES)
    mm_dt = F32R if USE_F32R else F32
    x1 = nc.dram_tensor("x1", [BC * H, W], F32, kind="ExternalInput")
    x2 = nc.dram_tensor("x2", [BC * H2, W2], mm_dt, kind="ExternalInput")
    out = nc.dram_tensor("out", [BC * H, W], F32, kind="ExternalOutput")

    # Row-interleaved views: partition p <-> (row % 128) so per-partition
    # scalars line up with output row blocks. x2/out grouped 2 (b,c) pairs
    # per DMA to halve DMA instruction count.
    x1v = x1.ap().rearrange("(g j p) w -> g p j w", j=NJ * X1_GRP, p=128)
    x2v = x2.ap().rearrange("(m c p) w -> m p c w", c=2 * NC2, p=128)
    outv = out.ap().rearrange("(m j p) w -> m p j w", j=2 * NJ, p=128)

    with TileContext(nc) as tc:
        with (
            tc.tile_pool(name="const", bufs=1) as cpool,
            tc.tile_pool(name="x1p", bufs=2) as x1pool,
            tc.tile_pool(name="rsp", bufs=2) as rspool,
            tc.tile_pool(name="x2p", bufs=3) as x2pool,
            tc.tile_pool(name="csb", bufs=6) as csbpool,
            tc.tile_pool(name="m2p", bufs=6) as m2pool,
            tc.tile_pool(name="op", bufs=6) as opool,
            tc.tile_pool(name="csp", bufs=4, space="PSUM") as cspool,
            tc.tile_pool(name="pbp", bufs=4, space="PSUM") as pbpool,
        ):
            ones_col = cpool.tile([128, 1], mm_dt)
            if USE_F32R:
                ones_f32 = cpool.tile([128, 1], F32)
                nc.vector.memset(ones_f32[:], 1.0)
                nc.vector.tensor_copy(ones_col[:], ones_f32[:])
            else:
                nc.vector.memset(ones_col[:], 1.0)
            scale_row = cpool.tile([1, 128], F32)
            nc.vector.memset(scale_row[:], SCALE)

            for _rep in range(reps):
              for g in range(N_GRP):
                # x1 rowsums for X1_GRP pairs at once.
                x1t = x1pool.tile([128, NJ * X1_GRP, W], F32)
                nc.sync.dma_start(out=x1t[:], in_=x1v[g])
                rs = rspool.tile([128, NJ * X1_GRP], F32)
                nc.vector.reduce_sum(out=rs[:], in_=x1t[:],
                                     axis=mybir.AxisListType.X)

                for s2 in range(X1_GRP // 2):
                    m = (g * X1_GRP) // 2 + s2
                    x2t = x2pool.tile([128, 2 * NC2, W2], mm_dt)
                    nc.sync.dma_start(out=x2t[:], in_=x2v[m])
                    ot = opool.tile([128, 2 * NJ, W], F32)

                    for k in range(2):  # the two (b,c) pairs in this load
                        # colsum2 over all 512 rows -> PSUM (1, 512)
                        cs = cspool.tile([1, W2], F32)
                        for ci in range(NC2):
                            nc.tensor.matmul(
                                cs[:],
                                lhsT=ones_col[:],
                                rhs=x2t[:, NC2 * k + ci, :],
                                start=(ci == 0),
                                stop=(ci == NC2 - 1),
                            )

                        # PSUM -> SBUF, then pair-add adjacent cols (avgpool).
                        csb = csbpool.tile([1, W2], F32)
                        nc.vector.tensor_copy(csb[:], cs[:])
                        m2 = m2pool.tile([1, W], F32)
                        csv = csb[:].rearrange("p (w t) -> p w t", t=2)
                        nc.vector.tensor_add(m2[:], csv[:, :, 0], csv[:, :, 1])

                        # Broadcast mean2 (scaled) to 128 partitions, K=1 mm.
                        pb = pbpool.tile([128, W], F32)
                        nc.tensor.matmul(
                            pb[:],
                            lhsT=scale_row[:],
                            rhs=m2[:],
                            start=True,
                            stop=True,
                        )

                        # Outer product: scale each partition by rowsum1.
                        for j in range(NJ):
                            col = NJ * (2 * s2 + k) + j
                            nc.scalar.activation(
                                ot[:, NJ * k + j, :], pb[:],
                                mybir.ActivationFunctionType.Copy,
                                scale=rs[:, col:col + 1],
                            )
                    # Store via the scalar engine's DGE queue so stores don't
                    # head-of-line block the SP queue that issues loads.
                    nc.scalar.dma_start(out=outv[m], in_=ot[:])

    nc.compile()
    _built[reps] = nc
    return nc


def _in_maps(x1, x2):
    x1 = np.ascontiguousarray(np.asarray(x1), dtype=np.float32)
    x2 = np.ascontiguousarray(np.asarray(x2), dtype=np.float32)
    maps = []
    for i in range(N_CORES):
        maps.append({
            "x1": x1[i * B_LOC:(i + 1) * B_LOC].reshape(BC * H, W),
            "x2": x2[i * B_LOC:(i + 1) * B_LOC].reshape(BC * H2, W2),
        })
    return maps


def _run(x1, x2, **kw):
    nc = _build()
    return run_bass_kernel_spmd(nc, _in_maps(x1, x2), list(range(N_CORES)), **kw)


def kernel(x1, x2):
    res = _run(x1, x2)
    outs = [res.results[i]["out"].reshape(B_LOC, C, H, W)
            for i in range(N_CORES)]
    return np.concatenate(outs, axis=0)
